# revision 1
# baseline (speedup 1.0000x reference)
"""CascadeAttention kernel — data-parallel across 8 NeuronCores.

Shards the window/batch dim B=128 across 8 cores (16 windows each, per the
sharding hint); all parameters are small and replicated. BN affine params and
the relative-position-bias gather are folded on the host (parameter-only
transforms); the per-window compute (qkv matmul, depthwise 3x3x3 conv,
attention softmax, projection) runs on the NeuronCores.
"""
import numpy as np
import jax
import jax.numpy as jnp

# Hardcoded problem shapes (nn_CascadeAttention_28063316312381)
WS = (8, 7, 7)
N = WS[0] * WS[1] * WS[2]          # 392 tokens per window
NUM_HEADS = 8
KEY_DIM = 16
D = 32                              # value dim per head
DIM = 256
B = 128
EPS = 1e-5
SCALE = KEY_DIM ** -0.5
NCORES = 8
BSH = B // NCORES                   # 16 windows per core


def _fold_bn(g, b, m, v):
    # inference batchnorm y = x*s + t with s = g/rsqrt(v+eps), t = b - m*s
    s = g / np.sqrt(v + EPS)
    t = b - m * s
    return s.astype(np.float32), t.astype(np.float32)


def _shard_fn(x, qkv_w_f, qkv_t, dw_w_f, dw_t, proj_w_f, proj_t, bias):
    # x: [BSH, DIM, d, h, w] one core's shard. All params replicated.
    Wd, Wh, Ww = WS
    xf = x.reshape(BSH, DIM, N)
    feats_in = jnp.split(xf, NUM_HEADS, axis=1)     # nh x [b, 32, N]
    feats_out = []
    feat = feats_in[0]
    for i in range(NUM_HEADS):
        if i > 0:
            feat = feat + feats_in[i]
        # folded 1x1x1 conv + BN: [64,32] @ [b,32,N] + t
        h = jnp.einsum('oi,bin->bon', qkv_w_f[i], feat) + qkv_t[i][None, :, None]
        q = h[:, :KEY_DIM]
        k = h[:, KEY_DIM:2 * KEY_DIM]
        v = h[:, 2 * KEY_DIM:]
        # depthwise 3x3x3 conv on q via 27 shifted MACs (BN folded into w/t)
        q3 = q.reshape(BSH, KEY_DIM, Wd, Wh, Ww)
        qp = jnp.pad(q3, ((0, 0), (0, 0), (1, 1), (1, 1), (1, 1)))
        acc = dw_t[i][None, :, None, None, None]
        acc = jnp.broadcast_to(acc, (BSH, KEY_DIM, Wd, Wh, Ww))
        for a in range(3):
            for bb in range(3):
                for c in range(3):
                    w_tap = dw_w_f[i, :, a, bb, c][None, :, None, None, None]
                    acc = acc + w_tap * qp[:, :, a:a + Wd, bb:bb + Wh, c:c + Ww]
        q = acc.reshape(BSH, KEY_DIM, N)
        # attention over N window tokens
        attn = jnp.einsum('bcn,bcm->bnm', q, k) * SCALE + bias[i][None]
        attn = jax.nn.softmax(attn, axis=-1)
        feat = jnp.einsum('bcm,bnm->bcn', v, attn)
        feats_out.append(feat)
    cat = jnp.concatenate(feats_out, axis=1)        # [b, 256, N]
    out = jnp.einsum('oi,bin->bon', proj_w_f, jax.nn.relu(cat))
    out = out + proj_t[None, :, None]
    return out.reshape(BSH, DIM, Wd, Wh, Ww)


_PMAPPED = None


def _get_pmapped():
    global _PMAPPED
    if _PMAPPED is None:
        _PMAPPED = jax.pmap(
            _shard_fn,
            in_axes=(0, None, None, None, None, None, None, None),
            devices=jax.devices()[:NCORES],
        )
    return _PMAPPED


def kernel(x, qkv_w, qkv_g, qkv_b, qkv_m, qkv_v, dw_w, dw_g, dw_b, dw_m, dw_v,
           proj_w, proj_g, proj_b, proj_m, proj_v, rpb, rel_index):
    x = np.asarray(x, dtype=np.float32)
    # --- host-side parameter folding (all tiny) ---
    qs, qt = _fold_bn(np.asarray(qkv_g), np.asarray(qkv_b),
                      np.asarray(qkv_m), np.asarray(qkv_v))       # [8,64]
    qkv_w_f = np.asarray(qkv_w) * qs[:, :, None]                   # [8,64,32]
    ds_, dt = _fold_bn(np.asarray(dw_g), np.asarray(dw_b),
                       np.asarray(dw_m), np.asarray(dw_v))         # [8,16]
    dw_w_f = (np.asarray(dw_w)[:, :, 0] * ds_[:, :, None, None, None])  # [8,16,3,3,3]
    ps, pt = _fold_bn(np.asarray(proj_g), np.asarray(proj_b),
                      np.asarray(proj_m), np.asarray(proj_v))      # [256]
    proj_w_f = np.asarray(proj_w) * ps[:, None]                    # [256,256]
    # relative position bias gather on host: [nh, N, N]
    rel = np.asarray(rel_index).reshape(-1)
    bias = np.asarray(rpb)[rel].reshape(N, N, NUM_HEADS).transpose(2, 0, 1)
    bias = np.ascontiguousarray(bias, dtype=np.float32)

    xs = x.reshape(NCORES, BSH, DIM, *WS)
    fn = _get_pmapped()
    out = fn(xs, jnp.asarray(qkv_w_f), jnp.asarray(qt), jnp.asarray(dw_w_f),
             jnp.asarray(dt), jnp.asarray(proj_w_f), jnp.asarray(pt),
             jnp.asarray(bias))
    out = np.asarray(out, dtype=np.float32).reshape(B, DIM, *WS)
    return out

